# revision 1
# baseline (speedup 1.0000x reference)
"""LookAheadMask kernel for Trainium2.

out[b, r, c] = 1.0 if c > r else x[b, r, c], for x of shape (8, 4096, 4096) f32.

Sharding: batch dim across 8 NeuronCores (data parallel, no communication).

Per-core plan (matrix is S x S, S=4096, row-blocks of P=128), raw bass
(explicit engines + semaphores; the Tile drain would exceed walrus's
sync-wait-slot limit with this many independent DMAs):

  - strictly-lower region (cols < block start): 31 direct DRAM->DRAM copies
  - strictly-upper region (cols >= block end): 31 DMAs from an SBUF ones
    tile (no HBM read for that half)
  - the 32 diagonal 128x128 blocks: one 3D-strided gather DMA into SBUF
    [128, 32*128], one gpsimd affine_select (keep x where
    row >= col-within-block, else 1.0), one scatter back.

A single HWDGE ring executes queued DMAs one at a time (profiled: slice
durations sum to the whole span), so the 62 bulk DMAs are split round-robin
across three descriptor paths that run concurrently: SP ring (sync), ACT
ring (scalar), and SWDGE (gpsimd). Row-block i's copy (i*64KB) and ones
((31-i)*64KB) pair to ~2MB, so assigning pairs round-robin balances bytes.

HBM traffic/core: ~33 MiB read + 64 MiB write vs 128 MiB naive.
"""

import numpy as np

from concourse import bass, mybir
from concourse.bass_utils import run_bass_kernel_spmd

S = 4096
P = 128
NB = S // P  # 32
N_CORES = 8

_cached_nc = None


def _build():
    global _cached_nc
    if _cached_nc is not None:
        return _cached_nc

    nc = bass.Bass()
    x = nc.dram_tensor("x", [S, S], mybir.dt.float32, kind="ExternalInput")
    out = nc.dram_tensor("out", [S, S], mybir.dt.float32, kind="ExternalOutput")

    # Diagonal-block view: [row-in-block(128), block(32), col-in-block(128)],
    # block b starts at element offset b*(P*S + P). Strides in elements.
    diag_pairs = [[S, P], [P * S + P, NB], [1, P]]
    # Gather window: W cols per diag block ending at its right edge, so
    # descriptors are W*4 bytes instead of 512 (the 512B-descriptor gather
    # profiled at 152us for 2MB). Blocks 1..31 in one DMA; block 0's window
    # would start before the tensor, so it gets its own 128-col load.
    W = 256

    # 62 bulk DMAs all HWDGE (dsem): 47 on the SP ring, 15 on the ACT ring
    # (issued after the ACT ring's cheap wide-window diag gather)

    def bulk(eng, blocks, ones):
        """Emit copy then ones DMAs for the given row-blocks on one engine."""
        for i in blocks:
            r0 = i * P
            if i > 0:
                eng.dma_start(
                    out=out[r0 : r0 + P, 0:r0], in_=x[r0 : r0 + P, 0:r0]
                ).then_inc(dsem, 16)
        eng.wait_ge(msem, 1)
        for i in blocks:
            r0 = i * P
            if i < NB - 1:
                w = S - r0 - P
                eng.dma_start(
                    out=out[r0 : r0 + P, r0 + P : S], in_=ones[:, :w]
                ).then_inc(dsem, 16)

    with (
        nc.Block() as block,
        nc.semaphore("dsem") as dsem,  # bulk DMA completions (HWDGE rings)
        nc.semaphore("gsem") as gsem,  # diag gather done
        nc.semaphore("ssem") as ssem,  # diag scatter done
        nc.semaphore("msem") as msem,  # ones memset done
        nc.semaphore("asem") as asem,  # affine_select done
        nc.sbuf_tensor("ones", [P, S], mybir.dt.float32) as ones,
        nc.sbuf_tensor("diag_in2", [P, NB * W], mybir.dt.float32) as diag_in2,
        nc.sbuf_tensor("diag_out", [P, S], mybir.dt.float32) as diag_out,
    ):

        @block.vector
        def _(vector: bass.BassVectorEngine):
            vector.memset(ones[:, :], 1.0).then_inc(msem, 1)

        @block.scalar
        def _(scalar: bass.BassEngine):
            scalar.dma_start(
                out=bass.AP(diag_in2, W, [[NB * W, P], [W, NB - 1], [1, W]]),
                in_=bass.AP(x, (P * S + P) + P - W, [[S, P], [P * S + P, NB - 1], [1, W]]),
            ).then_inc(gsem, 16)
            scalar.dma_start(
                out=bass.AP(diag_in2, W - P, [[NB * W, P], [1, P]]),
                in_=x[0:P, 0:P],
            ).then_inc(gsem, 16)
            bulk(scalar, range(3, NB, 4), ones)
            scalar.wait_ge(asem, 1)
            scalar.dma_start(
                out=bass.AP(out, 0, diag_pairs), in_=diag_out[:, :]
            ).then_inc(ssem, 16)

        @block.gpsimd
        def _(gpsimd: bass.BassGpSimd):
            gpsimd.wait_ge(gsem, 32)
            # iota[p, c] = p - (c % 128); keep x where >= 0 (at/below diag).
            # Input reads the last 128 cols of each W-wide gathered block.
            gpsimd.affine_select(
                out=diag_out[:, :],
                in_=bass.AP(diag_in2, W - P, [[NB * W, P], [W, NB], [1, P]]),
                pattern=[[0, NB], [-1, P]],
                base=0,
                channel_multiplier=1,
                compare_op=mybir.AluOpType.is_ge,
                fill=1.0,
            ).then_inc(asem, 1)

        @block.sync
        def _(sync: bass.BassEngine):
            bulk(sync, [i for i in range(NB) if i % 4 != 3], ones)
            sync.wait_ge(dsem, 16 * 62)
            sync.wait_ge(ssem, 16)

    _cached_nc = nc
    return nc


def _run(x_full: np.ndarray, trace: bool = False):
    nc = _build()
    x_full = np.asarray(x_full, dtype=np.float32)
    in_maps = [{"x": x_full[i]} for i in range(N_CORES)]
    res = run_bass_kernel_spmd(nc, in_maps, list(range(N_CORES)), trace=trace)
    out = np.stack([res.results[i]["out"] for i in range(N_CORES)], axis=0)
    return out, res


def kernel(x: np.ndarray) -> np.ndarray:
    out, _ = _run(x, trace=False)
    return out



# revision 4
# speedup vs baseline: 1.9334x; 1.9334x over previous
"""LookAheadMask kernel for Trainium2 — in-place variant.

out[b, r, c] = 1.0 if c > r else x[b, r, c], for x of shape (8, 4096, 4096) f32.

Sharding: batch dim across 8 NeuronCores (data parallel, no communication).

Per-core plan (matrix is S x S, S=4096, row-blocks of P=128): the output
aliases the input buffer (lowering_input_output_aliases={0: 0} through the
BIR-lowering/NKI path), so the strictly-lower triangle never moves at all.
The kernel only
  - writes ones into the strictly-upper region (31 row-block rectangles,
    sourced from an SBUF ones tile; ~31 MiB of HBM writes),
  - fixes the 32 diagonal 128x128 blocks: one 3D-strided gather DMA into
    SBUF (W=256-wide windows so descriptors are 1 KiB), one gpsimd
    affine_select (keep x where row >= col-within-block, else 1.0), one
    scatter back (2 MiB write).

HBM traffic/core: ~4 MiB read + ~33 MiB write vs 33 + 64 for the
copy-everything variant and 64 + 64 naive.

The 31+1 bulk DMAs are split across the two HWDGE rings (SP ring via the
sync engine, ACT ring via the scalar engine) greedily so both rings carry
roughly equal bytes; the scalar ring also carries the diag gather/scatter.
"""

import numpy as np

S = 4096
P = 128
NB = S // P  # 32
N_CORES = 8
W = 256  # diag gather window width (1 KiB descriptors)

_cached = None


def _build():
    from concourse import bass, mybir

    nc = bass.Bass(target_bir_lowering=True, enable_partition_id=False)
    x = nc.dram_tensor("x", [S, S], mybir.dt.float32, kind="ExternalInput")
    out = nc.dram_tensor("out", [S, S], mybir.dt.float32, kind="ExternalOutput")

    # Diagonal-block view: [row-in-block(128), block(32), col-in-block(128)],
    # block b starts at element offset b*(P*S + P). Strides in elements.
    diag_pairs = [[S, P], [P * S + P, NB], [1, P]]

    # Greedy byte-balance of the 31 ones rectangles across the two HWDGE
    # rings. Scalar starts preloaded with the diag traffic (4 MiB gather read
    # + 2 MiB scatter write = 96 units of 64 KiB).
    loads = {"scalar": 96, "sync": 0}
    rings: dict[str, list[int]] = {"scalar": [], "sync": []}
    for i in sorted(range(NB - 1), key=lambda i: NB - 1 - i, reverse=True):
        tgt = min(loads, key=lambda k: loads[k])
        rings[tgt].append(i)
        loads[tgt] += NB - 1 - i

    def ones_dmas(eng, blocks, ones, dsem, msem):
        eng.wait_ge(msem, 1)
        for i in blocks:
            r0 = i * P
            w = S - r0 - P
            eng.dma_start(
                out=out[r0 : r0 + P, r0 + P : S], in_=ones[:, :w]
            ).then_inc(dsem, 16)

    with (
        nc.Block() as block,
        nc.semaphore("dsem") as dsem,  # ones DMA completions
        nc.semaphore("gsem") as gsem,  # diag gather done
        nc.semaphore("ssem") as ssem,  # diag scatter done
        nc.semaphore("msem") as msem,  # ones memset done
        nc.semaphore("asem") as asem,  # affine_select done
        nc.sbuf_tensor("ones", [P, S - P], mybir.dt.float32) as ones,
        nc.sbuf_tensor("diag_in2", [P, NB * W], mybir.dt.float32) as diag_in2,
        nc.sbuf_tensor("diag_out", [P, S], mybir.dt.float32) as diag_out,
    ):

        @block.vector
        def _(vector: bass.BassVectorEngine):
            vector.memset(ones[:, :], 1.0).then_inc(msem, 1)

        @block.scalar
        def _(scalar: bass.BassEngine):
            # Gather window: W cols per diag block ending at its right edge.
            # Blocks 1..31 in one DMA; block 0's window would start before
            # the tensor, so it gets its own 128-col load.
            scalar.dma_start(
                out=bass.AP(diag_in2, W, [[NB * W, P], [W, NB - 1], [1, W]]),
                in_=bass.AP(
                    x, (P * S + P) + P - W, [[S, P], [P * S + P, NB - 1], [1, W]]
                ),
            ).then_inc(gsem, 16)
            scalar.dma_start(
                out=bass.AP(diag_in2, W - P, [[NB * W, P], [1, P]]),
                in_=x[0:P, 0:P],
            ).then_inc(gsem, 16)
            ones_dmas(scalar, rings["scalar"], ones, dsem, msem)
            scalar.wait_ge(asem, 1)
            scalar.dma_start(
                out=bass.AP(out, 0, diag_pairs), in_=diag_out[:, :]
            ).then_inc(ssem, 16)

        @block.gpsimd
        def _(gpsimd: bass.BassGpSimd):
            gpsimd.wait_ge(gsem, 32)
            # iota[p, c] = p - (c % 128); keep x where >= 0 (at/below diag).
            # Input reads the last 128 cols of each W-wide gathered window.
            gpsimd.affine_select(
                out=diag_out[:, :],
                in_=bass.AP(diag_in2, W - P, [[NB * W, P], [W, NB], [1, P]]),
                pattern=[[0, NB], [-1, P]],
                base=0,
                channel_multiplier=1,
                compare_op=mybir.AluOpType.is_ge,
                fill=1.0,
            ).then_inc(asem, 1)

        @block.sync
        def _(sync: bass.BassEngine):
            ones_dmas(sync, rings["sync"], ones, dsem, msem)
            sync.wait_ge(dsem, 16 * (NB - 1))
            sync.wait_ge(ssem, 16)

    nc.finalize()
    return nc


def _make_runner():
    """Compile-once runner: jit(shard_map(_body)) over 8 cores with the
    output aliased to the (donated) input — mirrors
    bass2jax.run_bass_via_pjrt, plus lowering_input_output_aliases."""
    global _cached
    if _cached is not None:
        return _cached

    import jax
    from jax.sharding import Mesh, PartitionSpec
    from jax.experimental.shard_map import shard_map
    from concourse import bass2jax

    bass2jax.install_neuronx_cc_hook()
    nc = _build()

    def _body(xg):
        outs = bass2jax._bass_exec_p.bind(
            xg,
            out_avals=(jax.core.ShapedArray((S, S), np.float32),),
            in_names=("x",),
            out_names=("out",),
            lowering_input_output_aliases=((0, 0),),
            sim_require_finite=True,
            sim_require_nnan=True,
            nc=nc,
        )
        return tuple(outs)

    devices = jax.devices()[:N_CORES]
    assert len(devices) == N_CORES, f"need {N_CORES} devices, have {len(devices)}"
    mesh = Mesh(np.asarray(devices), ("core",))
    sharded = jax.jit(
        shard_map(
            _body,
            mesh=mesh,
            in_specs=(PartitionSpec("core"),),
            out_specs=(PartitionSpec("core"),),
            check_rep=False,
        ),
        donate_argnums=(0,),
        keep_unused=True,
    )
    _cached = (nc, sharded)
    return _cached


class _Result:
    def __init__(self, exec_time_ns=None, mean_exec_time_ns=None):
        self.exec_time_ns = exec_time_ns
        self.mean_exec_time_ns = mean_exec_time_ns


def _run(x_full: np.ndarray, trace: bool = False):
    nc, sharded = _make_runner()
    x_full = np.asarray(x_full, dtype=np.float32)
    xg = np.ascontiguousarray(x_full.reshape(N_CORES * S, S))

    if not trace:
        out = sharded(xg)[0]
        return np.asarray(out).reshape(N_CORES, S, S), _Result()

    # Trace path (test.py only): NTFF profile around the execution, then the
    # same gauge/perfetto pipeline run_bass_kernel_spmd uses under axon.
    import glob
    import os
    import tempfile

    from antenv.axon_hooks import get_axon_ntff_profile_hook
    from concourse import bass_utils as BU

    neff_dir = tempfile.mkdtemp()
    hook = get_axon_ntff_profile_hook()
    with hook(neff_dir, [0]):
        out = np.asarray(sharded(xg)[0])

    ntffs = glob.glob(os.path.join(neff_dir, "*_body*.ntff"))
    if not ntffs:
        return out.reshape(N_CORES, S, S), _Result()

    sharepath = BU.upload_artifacts(neff_dir)
    profile = BU.gauge.profiler.Profile(
        profile_path=BU.FishPath(neff_dir),
        kernel_dev_mode=True,
        profile_on_exit=False,
        bass_kernel=nc.m,
        offline_processing=True,
        fname="*_body*",
        annotate_hlo=False,
        metadata={"artifacts_path": sharepath},
    )
    perf = BU._process_ntff_profile(
        profile,
        neff_dir,
        nc,
        list(range(N_CORES)),
        None,
        False,
        {},
        trace_events=False,
    )
    return out.reshape(N_CORES, S, S), _Result(
        perf.exec_time_ns, perf.mean_exec_time_ns
    )


def kernel(x: np.ndarray) -> np.ndarray:
    out, _ = _run(x, trace=False)
    return out


# revision 5
# speedup vs baseline: 1.9678x; 1.0178x over previous
"""LookAheadMask kernel for Trainium2 — in-place variant, pipelined diag.

out[b, r, c] = 1.0 if c > r else x[b, r, c], for x of shape (8, 4096, 4096) f32.

Sharding: batch dim across 8 NeuronCores (data parallel, no communication).

The output aliases the input buffer (lowering_input_output_aliases={0: 0}
through the BIR-lowering/NKI path), so the strictly-lower triangle never
moves. Per core the kernel only
  - writes ones into the strictly-upper region: 31 row-block rectangles
    [128 x w] from an SBUF ones tile (~31 MiB HBM writes, big descriptors,
    measured ~470 GB/s per HWDGE ring),
  - fixes the 32 diagonal 128x128 blocks. Small-descriptor DMAs cost
    ~7-10 ns/descriptor regardless of bytes, so the diag path is pipelined:
    the 1 KiB-descriptor gather is split in two chunks, one per HWDGE ring,
    issued first (16 us of ring time each); gpsimd then affine_selects each
    half and immediately issues its 512 B-descriptor scatter chunk on the
    SWDGE queue, keeping the scatters off the rings entirely so their drain
    overlaps the bulk ones writes.

HBM traffic/core: ~4 MiB read + ~33 MiB write (vs 128 MiB naive).
"""

import numpy as np

S = 4096
P = 128
NB = S // P  # 32
N_CORES = 8
W = 256  # diag gather window width (1 KiB descriptors)

_cached = None


def _build():
    from concourse import bass, mybir

    nc = bass.Bass(target_bir_lowering=True, enable_partition_id=False)
    x = nc.dram_tensor("x", [S, S], mybir.dt.float32, kind="ExternalInput")
    out = nc.dram_tensor("out", [S, S], mybir.dt.float32, kind="ExternalOutput")

    DB = P * S + P  # element stride between consecutive diagonal blocks

    # Greedy byte-balance of the 31 ones rectangles across the two HWDGE
    # rings, with each ring preloaded by its gather chunk's descriptor cost
    # (~8.3 ns/desc => ~115 units for 1920 descs, ~131 for 2176).
    loads = {"sync": 115, "scalar": 131}
    rings: dict[str, list[int]] = {"sync": [], "scalar": []}
    for i in range(NB - 1):  # block 0 is the widest rectangle
        tgt = min(loads, key=lambda k: loads[k])
        rings[tgt].append(i)
        loads[tgt] += NB - 1 - i

    def ones_dmas(eng, blocks, ones, dsem, msem):
        eng.wait_ge(msem, 1)
        for i in blocks:
            r0 = i * P
            w = S - r0 - P
            eng.dma_start(
                out=out[r0 : r0 + P, r0 + P : S], in_=ones[:, :w]
            ).then_inc(dsem, 16)

    with (
        nc.Block() as block,
        nc.semaphore("dsem") as dsem,  # ones DMA completions
        nc.semaphore("gsa") as gsa,  # gather chunk on sync ring done
        nc.semaphore("gsb") as gsb,  # block0 + gather chunk on scalar ring
        nc.semaphore("ssem") as ssem,  # diag scatters done
        nc.semaphore("msem") as msem,  # ones memset done
        nc.sbuf_tensor("ones", [P, S - P], mybir.dt.float32) as ones,
        nc.sbuf_tensor("diag_in2", [P, NB * W], mybir.dt.float32) as diag_in2,
        nc.sbuf_tensor("diag_out", [P, S], mybir.dt.float32) as diag_out,
    ):

        @block.vector
        def _(vector: bass.BassVectorEngine):
            vector.memset(ones[:, :], 1.0).then_inc(msem, 1)

        @block.sync
        def _(sync: bass.BassEngine):
            # Gather window: W cols per diag block ending at its right edge.
            # Blocks 1..15 here; 16..31 + block 0 on the scalar ring.
            sync.dma_start(
                out=bass.AP(diag_in2, W, [[NB * W, P], [W, 15], [1, W]]),
                in_=bass.AP(x, DB + P - W, [[S, P], [DB, 15], [1, W]]),
            ).then_inc(gsa, 16)
            ones_dmas(sync, rings["sync"], ones, dsem, msem)
            sync.wait_ge(dsem, 16 * (NB - 1))
            sync.wait_ge(ssem, 32)

        @block.scalar
        def _(scalar: bass.BassEngine):
            # Block 0's window would start before the tensor: own 128-col load.
            scalar.dma_start(
                out=bass.AP(diag_in2, W - P, [[NB * W, P], [1, P]]),
                in_=x[0:P, 0:P],
            ).then_inc(gsb, 16)
            scalar.dma_start(
                out=bass.AP(diag_in2, 16 * W, [[NB * W, P], [W, 16], [1, W]]),
                in_=bass.AP(x, 16 * DB + P - W, [[S, P], [DB, 16], [1, W]]),
            ).then_inc(gsb, 16)
            ones_dmas(scalar, rings["scalar"], ones, dsem, msem)

        @block.gpsimd
        def _(gpsimd: bass.BassGpSimd):
            # iota[p, c] = p - (c % 128); keep x where >= 0 (at/below diag).
            # Input reads the last 128 cols of each W-wide gathered window.
            # Each half: select then immediately scatter on the SWDGE queue
            # (engine program order sequences select -> dma).
            gpsimd.wait_ge(gsb, 16)  # block 0
            gpsimd.wait_ge(gsa, 16)  # blocks 1-15
            gpsimd.affine_select(
                out=diag_out[:, : 16 * P],
                in_=bass.AP(diag_in2, W - P, [[NB * W, P], [W, 16], [1, P]]),
                pattern=[[0, 16], [-1, P]],
                base=0,
                channel_multiplier=1,
                compare_op=mybir.AluOpType.is_ge,
                fill=1.0,
            )
            gpsimd.dma_start(
                out=bass.AP(out, 0, [[S, P], [DB, 16], [1, P]]),
                in_=diag_out[:, : 16 * P],
            ).then_inc(ssem, 16)
            gpsimd.wait_ge(gsb, 32)  # blocks 16-31
            gpsimd.affine_select(
                out=diag_out[:, 16 * P :],
                in_=bass.AP(
                    diag_in2, 16 * W + W - P, [[NB * W, P], [W, 16], [1, P]]
                ),
                pattern=[[0, 16], [-1, P]],
                base=0,
                channel_multiplier=1,
                compare_op=mybir.AluOpType.is_ge,
                fill=1.0,
            )
            gpsimd.dma_start(
                out=bass.AP(out, 16 * DB, [[S, P], [DB, 16], [1, P]]),
                in_=diag_out[:, 16 * P :],
            ).then_inc(ssem, 16)

    nc.finalize()
    return nc


def _make_runner():
    """Compile-once runner: jit(shard_map(_body)) over 8 cores with the
    output aliased to the (donated) input — mirrors
    bass2jax.run_bass_via_pjrt, plus lowering_input_output_aliases."""
    global _cached
    if _cached is not None:
        return _cached

    import jax
    from jax.sharding import Mesh, PartitionSpec
    from jax.experimental.shard_map import shard_map
    from concourse import bass2jax

    bass2jax.install_neuronx_cc_hook()
    nc = _build()

    def _body(xg):
        outs = bass2jax._bass_exec_p.bind(
            xg,
            out_avals=(jax.core.ShapedArray((S, S), np.float32),),
            in_names=("x",),
            out_names=("out",),
            lowering_input_output_aliases=((0, 0),),
            sim_require_finite=True,
            sim_require_nnan=True,
            nc=nc,
        )
        return tuple(outs)

    devices = jax.devices()[:N_CORES]
    assert len(devices) == N_CORES, f"need {N_CORES} devices, have {len(devices)}"
    mesh = Mesh(np.asarray(devices), ("core",))
    sharded = jax.jit(
        shard_map(
            _body,
            mesh=mesh,
            in_specs=(PartitionSpec("core"),),
            out_specs=(PartitionSpec("core"),),
            check_rep=False,
        ),
        donate_argnums=(0,),
        keep_unused=True,
    )
    _cached = (nc, sharded)
    return _cached


class _Result:
    def __init__(self, exec_time_ns=None, mean_exec_time_ns=None):
        self.exec_time_ns = exec_time_ns
        self.mean_exec_time_ns = mean_exec_time_ns


def _run(x_full: np.ndarray, trace: bool = False):
    nc, sharded = _make_runner()
    x_full = np.asarray(x_full, dtype=np.float32)
    xg = np.ascontiguousarray(x_full.reshape(N_CORES * S, S))

    if not trace:
        out = sharded(xg)[0]
        return np.asarray(out).reshape(N_CORES, S, S), _Result()

    # Trace path (test.py only): NTFF profile around the execution, then the
    # same gauge/perfetto pipeline run_bass_kernel_spmd uses under axon.
    import glob
    import os
    import tempfile

    from antenv.axon_hooks import get_axon_ntff_profile_hook
    from concourse import bass_utils as BU

    neff_dir = tempfile.mkdtemp()
    hook = get_axon_ntff_profile_hook()
    with hook(neff_dir, [0]):
        out = np.asarray(sharded(xg)[0])

    ntffs = glob.glob(os.path.join(neff_dir, "*_body*.ntff"))
    if not ntffs:
        return out.reshape(N_CORES, S, S), _Result()

    sharepath = BU.upload_artifacts(neff_dir)
    profile = BU.gauge.profiler.Profile(
        profile_path=BU.FishPath(neff_dir),
        kernel_dev_mode=True,
        profile_on_exit=False,
        bass_kernel=nc.m,
        offline_processing=True,
        fname="*_body*",
        annotate_hlo=False,
        metadata={"artifacts_path": sharepath},
    )
    perf = BU._process_ntff_profile(
        profile,
        neff_dir,
        nc,
        list(range(N_CORES)),
        None,
        False,
        {},
        trace_events=False,
    )
    return out.reshape(N_CORES, S, S), _Result(
        perf.exec_time_ns, perf.mean_exec_time_ns
    )


def kernel(x: np.ndarray) -> np.ndarray:
    out, _ = _run(x, trace=False)
    return out


# revision 6
# speedup vs baseline: 2.2748x; 1.1560x over previous
"""LookAheadMask kernel for Trainium2 — in-place, merged diag writes.

out[b, r, c] = 1.0 if c > r else x[b, r, c], for x of shape (8, 4096, 4096) f32.

Sharding: batch dim across 8 NeuronCores (data parallel, no communication).

The output aliases the input buffer (lowering_input_output_aliases={0: 0}
through the BIR-lowering/NKI path), so the strictly-lower triangle never
moves. Per-core work is ~4 MiB of HBM reads + ~34 MiB of HBM writes.

Measured DMA-head behavior (v1/v2 traces): big-descriptor writes stream at
~430-470 GB/s per HWDGE ring; small-descriptor DMAs are head-limited at
~6-8 ns/desc on the SP ring but ~23 ns/desc on the ACT ring, and a
512 B-descriptor scatter costs ~25-60 us wherever it runs. So:

  - The 1 KiB-descriptor diag gather (4096 descs, unavoidable: the diag
    band is 4096 scattered 512 B row segments) runs entirely on the SP
    ring, split in two chunks to pipeline the selects.
  - There is NO scatter. A [128, 32*1024] SBUF tile (diag_sel) is
    pre-memset to 1.0 (split across DVE and gpsimd); gpsimd affine_selects
    only the 128-wide diagonal columns of each block into it; each diag
    block then leaves SBUF as the leading 128 cols of a [128 x 1024]
    4 KiB-descriptor rectangle (byte-bound, not desc-bound).
  - Pure-ones rectangles cover cols >= blockstart+1024 from a [128, 3072]
    ones tile; blocks 24-31 are fully covered by the (clipped) merged
    rectangles.
  - Two pure-ones rectangles go through the gpsimd SWDGE queue to measure
    a third DMA head; the rest are balanced SP/ACT.
"""

import numpy as np

S = 4096
P = 128
NB = S // P  # 32
N_CORES = 8
W = 256  # diag gather window width (1 KiB descriptors)
MW = 1024  # merged diag-rectangle width (4 KiB descriptors)
DB = P * S + P  # element stride between consecutive diagonal blocks

SWDGE_BLOCKS = [12, 16]  # pure-ones rects issued on the SWDGE queue
SP_BLOCKS = [0]  # pure-ones rects on the SP ring
ACT_BLOCKS = [i for i in range(24) if i not in SWDGE_BLOCKS + SP_BLOCKS]

_cached = None


def _build():
    from concourse import bass, mybir

    nc = bass.Bass(target_bir_lowering=True, enable_partition_id=False)
    x = nc.dram_tensor("x", [S, S], mybir.dt.float32, kind="ExternalInput")
    out = nc.dram_tensor("out", [S, S], mybir.dt.float32, kind="ExternalOutput")

    N_WRITES = 24 + 2 + 8  # pure ones + merged chunks + clipped blocks

    def pure_ones(eng, blocks, ones, dsem):
        for i in blocks:
            r0 = i * P
            w = S - r0 - MW
            eng.dma_start(
                out=out[r0 : r0 + P, r0 + MW : S], in_=ones[:, :w]
            ).then_inc(dsem, 16)

    with (
        nc.Block() as block,
        nc.semaphore("dsem") as dsem,  # all output-write DMA completions
        nc.semaphore("gsa") as gsa,  # gather chunks (SP ring)
        nc.semaphore("msem") as msem,  # ones memset done
        nc.semaphore("m2") as m2,  # diag_sel DVE-half memset done
        nc.semaphore("asem") as asem,  # affine_selects done
        nc.sbuf_tensor("ones", [P, S - MW], mybir.dt.float32) as ones,
        nc.sbuf_tensor("diag_in2", [P, NB * W], mybir.dt.float32) as diag_in2,
        nc.sbuf_tensor("diag_sel", [P, NB * MW], mybir.dt.float32) as diag_sel,
    ):

        @block.vector
        def _(vector: bass.BassVectorEngine):
            vector.memset(ones[:, :], 1.0).then_inc(msem, 1)
            vector.memset(diag_sel[:, : 16 * MW], 1.0).then_inc(m2, 1)

        @block.sync
        def _(sync: bass.BassEngine):
            # Diag gather, 1 KiB descriptors, all on the fast SP head.
            # Block 0's window would start before the tensor: own 128-col load.
            sync.dma_start(
                out=bass.AP(diag_in2, W - P, [[NB * W, P], [1, P]]),
                in_=x[0:P, 0:P],
            ).then_inc(gsa, 16)
            sync.dma_start(
                out=bass.AP(diag_in2, W, [[NB * W, P], [W, 15], [1, W]]),
                in_=bass.AP(x, DB + P - W, [[S, P], [DB, 15], [1, W]]),
            ).then_inc(gsa, 16)
            sync.dma_start(
                out=bass.AP(diag_in2, 16 * W, [[NB * W, P], [W, 16], [1, W]]),
                in_=bass.AP(x, 16 * DB + P - W, [[S, P], [DB, 16], [1, W]]),
            ).then_inc(gsa, 16)
            sync.wait_ge(msem, 1)
            pure_ones(sync, SP_BLOCKS, ones, dsem)
            # Merged rectangles for diag blocks 0-15: [128 x 1024] each,
            # leading 128 cols are the selected diag, rest ones.
            sync.wait_ge(asem, 1)
            sync.dma_start(
                out=bass.AP(out, 0, [[S, P], [DB, 16], [1, MW]]),
                in_=bass.AP(diag_sel, 0, [[NB * MW, P], [MW, 16], [1, MW]]),
            ).then_inc(dsem, 16)
            sync.wait_ge(dsem, 16 * N_WRITES)

        @block.scalar
        def _(scalar: bass.BassEngine):
            scalar.wait_ge(msem, 1)
            pure_ones(scalar, ACT_BLOCKS, ones, dsem)
            scalar.wait_ge(asem, 2)
            # Merged rectangles for diag blocks 16-23.
            scalar.dma_start(
                out=bass.AP(out, 16 * DB, [[S, P], [DB, 8], [1, MW]]),
                in_=bass.AP(
                    diag_sel, 16 * MW, [[NB * MW, P], [MW, 8], [1, MW]]
                ),
            ).then_inc(dsem, 16)
            # Blocks 24-31: merged rect clipped at the right edge covers the
            # whole remaining row span [r0, S).
            for b in range(24, 32):
                r0 = b * P
                w = S - r0
                scalar.dma_start(
                    out=out[r0 : r0 + P, r0:S],
                    in_=bass.AP(diag_sel, b * MW, [[NB * MW, P], [1, w]]),
                ).then_inc(dsem, 16)

        @block.gpsimd
        def _(gpsimd: bass.BassGpSimd):
            gpsimd.memset(diag_sel[:, 16 * MW :], 1.0)
            # iota[p, c] = p - (c % 128); keep x where >= 0 (at/below diag).
            # Select ONLY the 128 diag cols of each 1024-wide window; the
            # other 896 cols stay at the memset 1.0.
            gpsimd.wait_ge(gsa, 32)  # block 0 + blocks 1-15
            gpsimd.wait_ge(m2, 1)
            gpsimd.affine_select(
                out=bass.AP(diag_sel, 0, [[NB * MW, P], [MW, 16], [1, P]]),
                in_=bass.AP(diag_in2, W - P, [[NB * W, P], [W, 16], [1, P]]),
                pattern=[[0, 16], [-1, P]],
                base=0,
                channel_multiplier=1,
                compare_op=mybir.AluOpType.is_ge,
                fill=1.0,
            ).then_inc(asem, 1)
            gpsimd.wait_ge(msem, 1)
            pure_ones(gpsimd, SWDGE_BLOCKS[:1], ones, dsem)
            gpsimd.wait_ge(gsa, 48)  # blocks 16-31
            gpsimd.affine_select(
                out=bass.AP(
                    diag_sel, 16 * MW, [[NB * MW, P], [MW, 16], [1, P]]
                ),
                in_=bass.AP(
                    diag_in2, 16 * W + W - P, [[NB * W, P], [W, 16], [1, P]]
                ),
                pattern=[[0, 16], [-1, P]],
                base=0,
                channel_multiplier=1,
                compare_op=mybir.AluOpType.is_ge,
                fill=1.0,
            ).then_inc(asem, 1)
            pure_ones(gpsimd, SWDGE_BLOCKS[1:], ones, dsem)

    nc.finalize()
    return nc


def _make_runner():
    """Compile-once runner: jit(shard_map(_body)) over 8 cores with the
    output aliased to the (donated) input — mirrors
    bass2jax.run_bass_via_pjrt, plus lowering_input_output_aliases."""
    global _cached
    if _cached is not None:
        return _cached

    import jax
    from jax.sharding import Mesh, PartitionSpec
    from jax.experimental.shard_map import shard_map
    from concourse import bass2jax

    bass2jax.install_neuronx_cc_hook()
    nc = _build()

    def _body(xg):
        outs = bass2jax._bass_exec_p.bind(
            xg,
            out_avals=(jax.core.ShapedArray((S, S), np.float32),),
            in_names=("x",),
            out_names=("out",),
            lowering_input_output_aliases=((0, 0),),
            sim_require_finite=True,
            sim_require_nnan=True,
            nc=nc,
        )
        return tuple(outs)

    devices = jax.devices()[:N_CORES]
    assert len(devices) == N_CORES, f"need {N_CORES} devices, have {len(devices)}"
    mesh = Mesh(np.asarray(devices), ("core",))
    sharded = jax.jit(
        shard_map(
            _body,
            mesh=mesh,
            in_specs=(PartitionSpec("core"),),
            out_specs=(PartitionSpec("core"),),
            check_rep=False,
        ),
        donate_argnums=(0,),
        keep_unused=True,
    )
    _cached = (nc, sharded)
    return _cached


class _Result:
    def __init__(self, exec_time_ns=None, mean_exec_time_ns=None):
        self.exec_time_ns = exec_time_ns
        self.mean_exec_time_ns = mean_exec_time_ns


def _run(x_full: np.ndarray, trace: bool = False):
    nc, sharded = _make_runner()
    x_full = np.asarray(x_full, dtype=np.float32)
    xg = np.ascontiguousarray(x_full.reshape(N_CORES * S, S))

    if not trace:
        out = sharded(xg)[0]
        return np.asarray(out).reshape(N_CORES, S, S), _Result()

    # Trace path (test.py only): NTFF profile around the execution, then the
    # same gauge/perfetto pipeline run_bass_kernel_spmd uses under axon.
    import glob
    import os
    import tempfile

    from antenv.axon_hooks import get_axon_ntff_profile_hook
    from concourse import bass_utils as BU

    neff_dir = tempfile.mkdtemp()
    hook = get_axon_ntff_profile_hook()
    with hook(neff_dir, [0]):
        out = np.asarray(sharded(xg)[0])

    ntffs = glob.glob(os.path.join(neff_dir, "*_body*.ntff"))
    if not ntffs:
        return out.reshape(N_CORES, S, S), _Result()

    sharepath = BU.upload_artifacts(neff_dir)
    profile = BU.gauge.profiler.Profile(
        profile_path=BU.FishPath(neff_dir),
        kernel_dev_mode=True,
        profile_on_exit=False,
        bass_kernel=nc.m,
        offline_processing=True,
        fname="*_body*",
        annotate_hlo=False,
        metadata={"artifacts_path": sharepath},
    )
    perf = BU._process_ntff_profile(
        profile,
        neff_dir,
        nc,
        list(range(N_CORES)),
        None,
        False,
        {},
        trace_events=False,
    )
    return out.reshape(N_CORES, S, S), _Result(
        perf.exec_time_ns, perf.mean_exec_time_ns
    )


def kernel(x: np.ndarray) -> np.ndarray:
    out, _ = _run(x, trace=False)
    return out
